# revision 22
# baseline (speedup 1.0000x reference)
"""GQA attention kernel for Trainium2, 8-core tensor-parallel over kv heads.

Reference computation (fp32):
  q  = query @ q_proj.T + q_bias      -> heads (g-major): dq = gi*H*D + hi*D + d
  kv = query @ kv_proj.T + kv_bias    -> per kv head hi: k = cols [hi*2D, hi*2D+D), v = next D
  attn = softmax(q k^T / sqrt(D));  out = (attn v) @ out_proj.T + out_bias

Sharding: 8 cores; core c handles kv head h0 = c//2 and 4 query-head groups
gis = [0..3] (c even) or [4..7] (c odd). Each core computes a full-shape
partial of the output (rank-256 contribution); host sums the 8 partials.

Schedule (v7 = v4 + fast final norm): the attention phase is ACT-bound (exp
of 33.5M scores/core ~= 285us at 128 lanes * 1.2GHz) while the PE idles
~40% inside it -- and those idle slivers trigger HAM K=4/8 half-clock
windows.  P2 emission therefore runs an "aux work queue" on the PE: after
each mc-step's scores+AV-A matmuls, up to 5 queued aux matmuls are emitted.
Aux items (all 1-psum-bank so scores 4 + AV-A 2 + aux 1 + transpose 1 = 8
banks):
  - deferred head-B AV passes (exp outputs staged in SBUF), per 512-col half
  - P1 projection nuggets for batch 1 (16-ec accumulation, 512 cols)
  - V' PE-transposes, interleaved so no thin-matmul cluster forms
  - P3 output-projection 512-col tiles for already-normalized rows
Normalization is per (pair, half) so its DMA-roundtrip chain never gates
more than one unit.  The LAST unit's normalization -- which gates the final
32 P3 tiles with nothing else left to hide it -- replaces the 4-DMA
denominator round trip (denombuf -> packed -> recipbuf -> stride-0
broadcast, ~10us of serial DMA latency in the v4 trace) with an all-local
chain: psum dn rows -> [1,2048] reciprocal+cast on DVE -> ones[1,64]
PE-matmul broadcast into psum rows 64:128 -> tensor_mul reading psum.
Startup interleaves weight-chunk DMAs with the first tchunk's qt loads so
the first matmul issues after ~2 small DMAs.
"""
import sys

sys.path.insert(0, "/opt/trn_rl_repo")

from collections import deque

import ml_dtypes
import numpy as np

import concourse.bass as bass
import concourse.mybir as mybir
import concourse.tile as tile
from concourse import bacc

H, G, D = 4, 8, 64
L, N, E = 2048, 2, 2048
T = N * L
P = 128
DQ = 256  # per-core q dim: 4 groups x 64
SCALE = float(D) ** -0.5
F32 = mybir.dt.float32
BF16 = mybir.dt.bfloat16


def pbcast(ap2d, p):
    """[1, F] AP -> [p, F] AP broadcast across partitions (stride 0)."""
    return bass.AP(tensor=ap2d.tensor, offset=ap2d.offset, ap=[[0, p]] + list(ap2d.ap[1:]))


class AuxQueue:
    """FIFO of ('mm'|'free', closure) events.  'mm' events are budgeted
    (one PE matmul each); 'free' events (DMAs, DVE evicts, allocs) are
    emitted alongside at no budget cost."""

    def __init__(self):
        self.q = deque()

    def push(self, events):
        self.q.extend(events)

    def push_front(self, events):
        self.q.extendleft(reversed(events))

    def pop(self, budget):
        n = 0
        while self.q and n < budget:
            kind, fn = self.q.popleft()
            fn()
            if kind == 'mm':
                n += 1

    def drain(self):
        while self.q:
            kind, fn = self.q.popleft()
            fn()


class AuxPool:
    """Round-robin over aux psum tags (1 bank each).  Items resolve the
    pool at event-execution time via this holder, so deferred events run
    against whichever block's pool is current."""

    def __init__(self, pool, tags):
        self.pool = pool
        self.tags = tags
        self.i = 0

    def tile(self):
        t = self.pool.tile([P, 512], F32, name="aux", tag=self.tags[self.i])
        self.i = (self.i + 1) % len(self.tags)
        return t


CUR = {}  # CUR['apool'] = the active AuxPool


def build_nc():
    nc = bacc.Bacc("TRN2", target_bir_lowering=False, debug=False)
    add = mybir.AluOpType.add

    qT = nc.dram_tensor("qT", [E, T], BF16, kind="ExternalInput").ap()
    qpT = nc.dram_tensor("qpT", [E, DQ], BF16, kind="ExternalInput").ap()
    kvpT = nc.dram_tensor("kvpT", [E, P], BF16, kind="ExternalInput").ap()
    opT = nc.dram_tensor("opT", [DQ, E], BF16, kind="ExternalInput").ap()
    qb = nc.dram_tensor("qb", [P, 2], F32, kind="ExternalInput").ap()
    kvb = nc.dram_tensor("kvb", [P, 1], F32, kind="ExternalInput").ap()
    ident = nc.dram_tensor("ident", [P, P], BF16, kind="ExternalInput").ap()
    ones16 = nc.dram_tensor("ones16", [P, 16], BF16, kind="ExternalInput").ap()
    out = nc.dram_tensor("out", [T, E], BF16, kind="ExternalOutput").ap()
    denombuf = nc.dram_tensor("denombuf", [1, 8 * 2048], F32, kind="Internal").ap()
    recipbuf = nc.dram_tensor("recipbuf", [1, 8 * 2048], BF16, kind="Internal").ap()

    with tile.TileContext(nc) as tc, tc.tile_pool(name="data", bufs=1) as data, \
            tc.tile_pool(name="consts", bufs=1) as consts, \
            tc.tile_pool(name="qload", bufs=6) as qload, \
            tc.tile_pool(name="qload2", bufs=20) as qload2, \
            tc.tile_pool(name="expA", bufs=4) as expA, \
            tc.tile_pool(name="expB", bufs=26) as expB, \
            tc.tile_pool(name="scratch", bufs=3) as scratch, \
            tc.tile_pool(name="ostage", bufs=6) as ostage:
        qpT_c = [consts.tile([P, DQ], BF16, name=f"qp{ec}", tag=f"qp{ec}")
                 for ec in range(16)]
        kvpT_c = [consts.tile([P, P], BF16, name=f"kvp{ec}", tag=f"kvp{ec}")
                  for ec in range(16)]

        def load_weights(ec):
            nc.gpsimd.dma_start(out=kvpT_c[ec][:], in_=kvpT[ec * P:(ec + 1) * P, :])
            nc.gpsimd.dma_start(out=qpT_c[ec][:], in_=qpT[ec * P:(ec + 1) * P, :])

        load_weights(0)
        qb_sb = consts.tile([P, 2], F32)
        nc.sync.dma_start(out=qb_sb[:], in_=qb)
        kvb_sb = consts.tile([P, 1], F32)
        nc.sync.dma_start(out=kvb_sb[:], in_=kvb)

        QT0 = data.tile([P, T], BF16)  # dq 0:128   (gi_loc 0, 1)
        QT1 = data.tile([P, T], BF16)  # dq 128:256 (gi_loc 2, 3)
        KVT = data.tile([P, T], BF16)  # k rows 0:64, v rows 64:128
        KTdup = data.tile([P, T], BF16)  # k rows duplicated at partitions 64:128
        attn0 = data.tile([P, T], BF16)  # attnoutT c-chunk 0 (gi_loc 0, 1)
        attn1 = data.tile([P, T], BF16)  # c-chunk 1 (gi_loc 2, 3)
        Vp = [data.tile([P, 16 * 65], BF16, name=f"vp{n}", tag=f"vp{n}") for n in range(N)]
        Vtmp = data.tile([64, T], BF16)
        opT_sb = [data.tile([P, E], BF16, name=f"opt{cc}", tag=f"opt{cc}") for cc in range(2)]
        identb = consts.tile([P, P], BF16)

        def late_consts():
            """Small consts needed by transposes/AV; issued mid-tchunk0.
            The 1MB opT load is deferred past all weight chunks so it never
            delays the head's weight feed (which stalls head matmuls, lets
            the scheduler hoist aux work, and pushes the evictions the
            first scores' psum banks WAR on ~35us late)."""
            nc.sync.dma_start(out=identb[:], in_=ident)
            for n in range(N):
                vcol = Vp[n].rearrange("p (m c) -> p m c", c=65)[:, :, 64:65]
                nc.sync.dma_start(out=vcol, in_=ones16)

        def load_opT():
            for cc in range(2):
                nc.sync.dma_start(out=opT_sb[cc][:], in_=opT[cc * P:(cc + 1) * P, :])

        # ---------------- Phase 1 bulk (batch 0) ----------------
        def p1_tchunk(tchunk, trans, ps1, psT, hook=None):
            tcols = slice(tchunk * 1024, (tchunk + 1) * 1024)
            pq0 = ps1.tile([P, 1024], F32, tag="pq0")
            pkv = ps1.tile([P, 1024], F32, tag="pkv")
            pq1 = ps1.tile([P, 1024], F32, tag="pq1")
            tj = 0
            for ec in range(16):
                qt = qload.tile([P, 1024], BF16, tag="qt")
                nc.gpsimd.dma_start(out=qt[:], in_=qT[ec * P:(ec + 1) * P, tcols])
                if hook is not None:
                    hook(ec)
                first, last = ec == 0, ec == 15
                for ps_t, w in ((pq0, qpT_c[ec][:, 0:P]),
                                (pq1, qpT_c[ec][:, P:DQ]),
                                (pkv, kvpT_c[ec][:])):
                    for lq in range(2):
                        nc.tensor.matmul(ps_t[:, lq * 512:(lq + 1) * 512], lhsT=w,
                                         rhs=qt[:, lq * 512:(lq + 1) * 512],
                                         start=first, stop=last)
                if ec % 2 == 1 and tj < len(trans):
                    tn, tmc = trans[tj]
                    tj += 1
                    do_transpose(tn, tmc, psT)
            nc.vector.tensor_scalar(QT0[:, tcols], pq0[:], qb_sb[:, 0:1], None, op0=add)
            nc.vector.tensor_scalar(QT1[:, tcols], pq1[:], qb_sb[:, 1:2], None, op0=add)
            nc.vector.tensor_scalar(KVT[:, tcols], pkv[:], kvb_sb[:, 0:1], None, op0=add)
            nc.sync.dma_start(out=KTdup[64:128, tcols], in_=KVT[0:64, tcols])
            nc.sync.dma_start(out=Vtmp[0:64, tcols], in_=KVT[64:128, tcols])

        def do_transpose(n, mc, psT):
            pt = psT.tile([P, 64], BF16, name="pt", tag="pt")
            nc.tensor.transpose(pt[:], Vtmp[0:64, n * L + mc * P:n * L + (mc + 1) * P],
                                identb[0:64, 0:64])
            nc.vector.tensor_copy(Vp[n][:, mc * 65:mc * 65 + 64], pt[:])

        # ---------------- aux items ----------------
        def item_p1_super(tchunk, colhalf):
            """All three projection targets over one 512-col span, sharing a
            single set of 16 qt loads (issued on the idle gpsimd queue).
            Order pkv, pq0, pq1 so KTdup/Vtmp are produced earliest."""
            lo = tchunk * 1024 + colhalf * 512
            qts = {}

            def ldq(ec):
                qts[ec] = qload2.tile([P, 512], BF16, name="qt2", tag="qt2")
                nc.gpsimd.dma_start(out=qts[ec][:],
                                    in_=qT[ec * P:(ec + 1) * P, lo:lo + 512])
            ev = []
            for ec in range(6):
                ev.append(('free', lambda ec=ec: ldq(ec)))
            for ti, target in enumerate((2, 0, 1)):
                cell = {}

                def alloc(cell=cell):
                    cell['t'] = CUR['apool'].tile()
                ev.append(('free', alloc))
                for ec in range(16):
                    if ti == 0 and ec + 6 < 16:
                        ev.append(('free', lambda ec=ec: ldq(ec + 6)))

                    def mm(ec=ec, target=target, cell=cell, last=ti == 2):
                        w = (qpT_c[ec][:, 0:P], qpT_c[ec][:, P:DQ],
                             kvpT_c[ec][:])[target]
                        q = qts.pop(ec) if last else qts[ec]
                        nc.tensor.matmul(cell['t'][:], lhsT=w, rhs=q[:],
                                         start=ec == 0, stop=ec == 15)
                    ev.append(('mm', mm))

                def evict(cell=cell, target=target):
                    dst = (QT0, QT1, KVT)[target]
                    bias = (qb_sb[:, 0:1], qb_sb[:, 1:2], kvb_sb[:, 0:1])[target]
                    nc.vector.tensor_scalar(dst[:, lo:lo + 512], cell['t'][:],
                                            bias, None, op0=add)
                    if target == 2:
                        nc.sync.dma_start(out=KTdup[64:128, lo:lo + 512],
                                          in_=KVT[0:64, lo:lo + 512])
                        nc.sync.dma_start(out=Vtmp[0:64, lo:lo + 512],
                                          in_=KVT[64:128, lo:lo + 512])
                ev.append(('free', evict))
            return ev

        def item_avB(n, pair, half, lq, ebs):
            """Deferred head-B AV over staged exp tiles, one 512-col pass."""
            attnp = attn0 if pair == 0 else attn1
            lo = n * L + half * 1024 + lq * 512
            seg = ((n * 2 + pair) * 2 + half) * 2048
            cell = {}

            def alloc():
                cell['t'] = CUR['apool'].tile()
            ev = [('free', alloc)]
            for mc in range(16):
                def mm(mc=mc):
                    vw = Vp[n][:, mc * 65:mc * 65 + 65]
                    nc.tensor.matmul(cell['t'][0:65, :], lhsT=vw,
                                     rhs=ebs[mc][:, lq * 512:(lq + 1) * 512],
                                     start=mc == 0, stop=mc == 15)
                ev.append(('mm', mm))

            def evict():
                sc = scratch.tile([64, 512], BF16, name="sc", tag="sc")
                nc.vector.tensor_copy(sc[:], cell['t'][0:64, :])
                nc.sync.dma_start(out=attnp[64:128, lo:lo + 512], in_=sc[:])
                dnB = scratch.tile([1, 512], F32, name="dnB", tag="dnB")
                nc.vector.tensor_copy(dnB[:], cell['t'][64:65, :])
                dB = seg + 1024 + lq * 512
                nc.sync.dma_start(out=denombuf[0:1, dB:dB + 512], in_=dnB[:])
            ev.append(('free', evict))
            return ev

        def item_p3(tt, eo, lq, evict_eng='dve'):
            """Output-projection 512-col tile: rows tt*128, e cols eo*1024+lq*512."""
            trows = slice(tt * P, (tt + 1) * P)
            ecol = eo * 1024 + lq * 512
            cell = {}

            def alloc():
                cell['t'] = CUR['apool'].tile()
            ev = [('free', alloc)]
            for cc in range(2):
                def mm(cc=cc):
                    src = attn0 if cc == 0 else attn1
                    nc.tensor.matmul(cell['t'][:], lhsT=src[:, trows],
                                     rhs=opT_sb[cc][:, ecol:ecol + 512],
                                     start=cc == 0, stop=cc == 1)
                ev.append(('mm', mm))

            def evict():
                ost = ostage.tile([P, 512], BF16, name="ost", tag="ost")
                if evict_eng == 'dve':
                    nc.vector.tensor_copy(ost[:], cell['t'][:])
                else:
                    nc.scalar.copy(ost[:], cell['t'][:])
                nc.sync.dma_start(out=out[trows, ecol:ecol + 512], in_=ost[:])
            ev.append(('free', evict))
            return ev

        def item_transposes(n, mcs, psT):
            return [('mm', lambda mc=mc: do_transpose(n, mc, psT)) for mc in mcs]

        def weave(base, extra, every=4):
            """Interleave `extra` events into `base`, one per `every` base
            events, so thin transposes never cluster on the PE."""
            outev, j = [], 0
            for k, e in enumerate(base):
                outev.append(e)
                if k % every == every - 1 and j < len(extra):
                    outev.append(extra[j])
                    j += 1
            outev.extend(extra[j:])
            return outev

        def ev_norm(n, pair, half):
            """Normalize attn rows for one (pair, half): 1024 l cols."""
            def norm():
                attnp = attn0 if pair == 0 else attn1
                lo = n * L + half * 1024
                seg = ((n * 2 + pair) * 2 + half) * 2048
                packed = scratch.tile([P, 16], F32, name="packed", tag="packed")
                nc.sync.dma_start(
                    out=packed[:],
                    in_=denombuf[0:1, seg:seg + 2048].rearrange("a (p c) -> (a p) c", p=P))
                recp = scratch.tile([P, 16], F32, name="recp", tag="recp")
                nc.vector.reciprocal(recp[:], packed[:])
                recb = scratch.tile([P, 16], BF16, name="recb", tag="recb")
                nc.vector.tensor_copy(recb[:], recp[:])
                nc.sync.dma_start(
                    out=recipbuf[0:1, seg:seg + 2048].rearrange("a (p c) -> (a p) c", p=P),
                    in_=recb[:])
                bct = scratch.tile([P, 1024], BF16, name="bct", tag="bct")
                nc.sync.dma_start(out=bct[0:64, :],
                                  in_=pbcast(recipbuf[0:1, seg:seg + 1024], 64))
                nc.sync.dma_start(out=bct[64:128, :],
                                  in_=pbcast(recipbuf[0:1, seg + 1024:seg + 2048], 64))
                nc.vector.tensor_mul(attnp[:, lo:lo + 1024], attnp[:, lo:lo + 1024],
                                     bct[:])
            return [('free', norm)]

        # ---------------- Phase 2 unit with aux interleave ----------------
        def p2_unit(n, pair, half, ps_s, ps_avA, aux, sink=None):
            QTp = QT0 if pair == 0 else QT1
            attnp = attn0 if pair == 0 else attn1
            lo = n * L + half * 1024
            seg = ((n * 2 + pair) * 2 + half) * 2048
            avA = ps_avA.tile([65, 1024], F32, tag="avA")
            ebs = []
            for mc in range(16):
                mo = n * L + mc * P
                sA = ps_s.tile([P, 1024], F32, tag="sA")
                sB = ps_s.tile([P, 1024], F32, tag="sB")
                for lq in range(2):
                    lc2 = slice(lo + lq * 512, lo + lq * 512 + 512)
                    nc.tensor.matmul(sA[:, lq * 512:(lq + 1) * 512],
                                     lhsT=KVT[0:64, mo:mo + P],
                                     rhs=QTp[0:64, lc2])
                    nc.tensor.matmul(sB[:, lq * 512:(lq + 1) * 512],
                                     lhsT=KTdup[64:128, mo:mo + P],
                                     rhs=QTp[64:128, lc2])
                eA = expA.tile([P, 1024], BF16, tag="eA")
                eB = expB.tile([P, 1024], BF16, tag="eB")
                nc.scalar.activation(eA[:], sA[:], mybir.ActivationFunctionType.Exp,
                                     scale=SCALE)
                nc.scalar.activation(eB[:], sB[:], mybir.ActivationFunctionType.Exp,
                                     scale=SCALE)
                ebs.append(eB)
                vw = Vp[n][:, mc * 65:mc * 65 + 65]
                for lq in range(2):
                    nc.tensor.matmul(avA[:, lq * 512:(lq + 1) * 512], lhsT=vw,
                                     rhs=eA[:, lq * 512:(lq + 1) * 512],
                                     start=mc == 0, stop=mc == 15)
                aux.pop(5)
            # head-A rows + denominator evict; head-B AV deferred to aux
            nc.vector.tensor_copy(attnp[0:64, lo:lo + 1024], avA[0:64, :])
            dnA = scratch.tile([1, 1024], F32, tag="dnA")
            nc.vector.tensor_copy(dnA[:], avA[64:65, :])
            nc.sync.dma_start(out=denombuf[0:1, seg:seg + 1024], in_=dnA[:])
            ev = (item_avB(n, pair, half, 0, ebs)
                  + item_avB(n, pair, half, 1, ebs)
                  + ev_norm(n, pair, half))
            if sink is None:
                aux.push_front(ev)
            else:
                sink.extend(ev)

        # ================= schedule =================
        aux = AuxQueue()
        with tc.tile_pool(name="ps1", bufs=1, space="PSUM") as ps1, \
                tc.tile_pool(name="psT0", bufs=2, space="PSUM") as psT0:
            p1_tchunk(0, [], ps1, psT0,
                      hook=lambda ec: (load_weights(ec + 1) if ec <= 14 else None,
                                       late_consts() if ec == 6 else None))
            p1_tchunk(1, [(0, mc) for mc in range(8)], ps1, psT0)
            load_opT()

        pending = []  # events deferred from n0's last unit into n1's queue
        for n in range(N):
            with tc.tile_pool(name=f"ps_s{n}", bufs=1, space="PSUM") as ps_s, \
                    tc.tile_pool(name=f"ps_avA{n}", bufs=1, space="PSUM") as ps_avA, \
                    tc.tile_pool(name=f"ps_aux{n}", bufs=1, space="PSUM") as psx, \
                    tc.tile_pool(name=f"psT_{n}", bufs=1, space="PSUM") as psT:
                if n == 0:
                    CUR['apool'] = AuxPool(psx, ["aux"])
                    # batch-1 projections as 4 super-nuggets; n0's leftover
                    # transposes weave into the first (avA needs them ~step 8)
                    # and each span's V' transposes weave into the next
                    tq = item_transposes(0, range(8, 16), psT)
                    for tchunk in (2, 3):
                        for colhalf in range(2):
                            aux.push(weave(item_p1_super(tchunk, colhalf), tq,
                                           every=2 if tchunk == 2 and
                                           colhalf == 0 else 8))
                            mc0 = (tchunk - 2) * 8 + colhalf * 4
                            tq = item_transposes(1, range(mc0, mc0 + 4), psT)
                    aux.push(tq)
                else:
                    CUR['apool'] = AuxPool(psx, ["aux", "aux2"])
                    aux.push(pending)
                    # P3 for batch 0 rides the queue inside P2(n1)
                    for tt in range(16):
                        for eo in range(2):
                            for lq in range(2):
                                aux.push(item_p3(tt, eo, lq))
                for pair in range(2):
                    for half in range(2):
                        last = n == 0 and pair == 1 and half == 1
                        p2_unit(n, pair, half, ps_s, ps_avA, aux,
                                sink=pending if last else None)
                        if n == 1 and pair == 1 and half == 0:
                            # first half of P3(n1) can ride once its rows norm
                            for tt in range(16, 22):
                                for eo in range(2):
                                    for lq in range(2):
                                        aux.push(item_p3(tt, eo, lq))
                aux.drain()
                if n == 1:
                    # tail: remaining P3 rows, same pools (no boundary gap)
                    for tt in range(22, 32):
                        for eo in range(2):
                            for lq in range(2):
                                for kind, fn in item_p3(tt, eo, lq,
                                                        'dve' if lq == 0
                                                        else 'act'):
                                    fn()

    nc.compile()
    return nc


_NC_CACHE = None


def _get_nc():
    global _NC_CACHE
    if _NC_CACHE is None:
        _NC_CACHE = build_nc()
    return _NC_CACHE


def make_in_maps(query, q_proj, q_bias, kv_proj, kv_bias, out_proj):
    """Host-side sharding. Returns list of 8 per-core input dicts."""
    qT_h = np.ascontiguousarray(
        np.asarray(query, dtype=np.float32).transpose(2, 1, 0).reshape(E, T)
    ).astype(ml_dtypes.bfloat16)
    q_proj = np.asarray(q_proj, dtype=np.float32)
    q_bias = np.asarray(q_bias, dtype=np.float32)
    kv_proj = np.asarray(kv_proj, dtype=np.float32)
    kv_bias = np.asarray(kv_bias, dtype=np.float32)
    out_proj = np.asarray(out_proj, dtype=np.float32)
    ident = np.eye(P, dtype=np.float32)

    in_maps = []
    for c in range(8):
        h0 = c // 2
        gis = range(4) if c % 2 == 0 else range(4, 8)
        rows_q = np.array([gi * (H * D) + h0 * D + d for gi in gis for d in range(D)])
        kv_rows = slice(h0 * 2 * D, (h0 + 1) * 2 * D)
        in_maps.append({
            "qT": qT_h,
            "qpT": np.ascontiguousarray(q_proj[rows_q, :].T).astype(ml_dtypes.bfloat16),
            "kvpT": np.ascontiguousarray(kv_proj[kv_rows, :].T).astype(ml_dtypes.bfloat16),
            "opT": np.ascontiguousarray(out_proj[:, rows_q].T).astype(ml_dtypes.bfloat16),
            "qb": np.ascontiguousarray(q_bias[rows_q].reshape(2, P).T),
            "kvb": np.ascontiguousarray(kv_bias[kv_rows].reshape(P, 1)),
            "ident": ident.astype(ml_dtypes.bfloat16),
            "ones16": np.ones((P, 16), dtype=ml_dtypes.bfloat16),
        })
    return in_maps


def kernel(query, q_proj, q_bias, kv_proj, kv_bias, out_proj, out_bias):
    from concourse.bass_utils import run_bass_kernel_spmd

    nc = _get_nc()
    in_maps = make_in_maps(query, q_proj, q_bias, kv_proj, kv_bias, out_proj)
    res = run_bass_kernel_spmd(nc, in_maps, core_ids=list(range(8)))
    total = np.zeros((T, E), dtype=np.float64)
    for rmap in res.results:
        total += rmap["out"].astype(np.float64)
    total += np.asarray(out_bias, dtype=np.float64)[None, :]
    return np.ascontiguousarray(
        total.reshape(N, L, E).transpose(1, 0, 2)).astype(np.float32)


# revision 23
# speedup vs baseline: 1.0044x; 1.0044x over previous
"""GQA attention kernel for Trainium2, 8-core tensor-parallel over kv heads.

Reference computation (fp32):
  q  = query @ q_proj.T + q_bias      -> heads (g-major): dq = gi*H*D + hi*D + d
  kv = query @ kv_proj.T + kv_bias    -> per kv head hi: k = cols [hi*2D, hi*2D+D), v = next D
  attn = softmax(q k^T / sqrt(D));  out = (attn v) @ out_proj.T + out_bias

Sharding: 8 cores; core c handles kv head h0 = c//2 and 4 query-head groups
gis = [0..3] (c even) or [4..7] (c odd). Each core computes a full-shape
partial of the output (rank-256 contribution); host sums the 8 partials.

Schedule (v7 = v4 + fast final norm): the attention phase is ACT-bound (exp
of 33.5M scores/core ~= 285us at 128 lanes * 1.2GHz) while the PE idles
~40% inside it -- and those idle slivers trigger HAM K=4/8 half-clock
windows.  P2 emission therefore runs an "aux work queue" on the PE: after
each mc-step's scores+AV-A matmuls, up to 5 queued aux matmuls are emitted.
Aux items (all 1-psum-bank so scores 4 + AV-A 2 + aux 1 + transpose 1 = 8
banks):
  - deferred head-B AV passes (exp outputs staged in SBUF), per 512-col half
  - P1 projection nuggets for batch 1 (16-ec accumulation, 512 cols)
  - V' PE-transposes, interleaved so no thin-matmul cluster forms
  - P3 output-projection 512-col tiles for already-normalized rows
Normalization is per (pair, half) so its DMA-roundtrip chain never gates
more than one unit.  The LAST unit's normalization -- which gates the final
32 P3 tiles with nothing else left to hide it -- replaces the 4-DMA
denominator round trip (denombuf -> packed -> recipbuf -> stride-0
broadcast, ~10us of serial DMA latency in the v4 trace) with an all-local
chain: psum dn rows -> [1,2048] reciprocal+cast on DVE -> ones[1,64]
PE-matmul broadcast into psum rows 64:128 -> tensor_mul reading psum.
Startup interleaves weight-chunk DMAs with the first tchunk's qt loads so
the first matmul issues after ~2 small DMAs.
"""
import sys

sys.path.insert(0, "/opt/trn_rl_repo")

from collections import deque

import ml_dtypes
import numpy as np

import concourse.bass as bass
import concourse.mybir as mybir
import concourse.tile as tile
from concourse import bacc

H, G, D = 4, 8, 64
L, N, E = 2048, 2, 2048
T = N * L
P = 128
DQ = 256  # per-core q dim: 4 groups x 64
SCALE = float(D) ** -0.5
F32 = mybir.dt.float32
BF16 = mybir.dt.bfloat16


def pbcast(ap2d, p):
    """[1, F] AP -> [p, F] AP broadcast across partitions (stride 0)."""
    return bass.AP(tensor=ap2d.tensor, offset=ap2d.offset, ap=[[0, p]] + list(ap2d.ap[1:]))


class AuxQueue:
    """FIFO of ('mm'|'free', closure) events.  'mm' events are budgeted
    (one PE matmul each); 'free' events (DMAs, DVE evicts, allocs) are
    emitted alongside at no budget cost."""

    def __init__(self):
        self.q = deque()

    def push(self, events):
        self.q.extend(events)

    def push_front(self, events):
        self.q.extendleft(reversed(events))

    def pop(self, budget):
        n = 0
        while self.q and n < budget:
            kind, fn = self.q.popleft()
            fn()
            if kind == 'mm':
                n += 1

    def drain(self):
        while self.q:
            kind, fn = self.q.popleft()
            fn()


class AuxPool:
    """Round-robin over aux psum tags (1 bank each).  Items resolve the
    pool at event-execution time via this holder, so deferred events run
    against whichever block's pool is current."""

    def __init__(self, pool, tags):
        self.pool = pool
        self.tags = tags
        self.i = 0

    def tile(self):
        t = self.pool.tile([P, 512], F32, name="aux", tag=self.tags[self.i])
        self.i = (self.i + 1) % len(self.tags)
        return t


CUR = {}  # CUR['apool'] = the active AuxPool


def build_nc():
    nc = bacc.Bacc("TRN2", target_bir_lowering=False, debug=False)
    add = mybir.AluOpType.add

    qT = nc.dram_tensor("qT", [E, T], BF16, kind="ExternalInput").ap()
    qpT = nc.dram_tensor("qpT", [E, DQ], BF16, kind="ExternalInput").ap()
    kvpT = nc.dram_tensor("kvpT", [E, P], BF16, kind="ExternalInput").ap()
    opT = nc.dram_tensor("opT", [DQ, E], BF16, kind="ExternalInput").ap()
    qb = nc.dram_tensor("qb", [P, 2], F32, kind="ExternalInput").ap()
    kvb = nc.dram_tensor("kvb", [P, 1], F32, kind="ExternalInput").ap()
    ident = nc.dram_tensor("ident", [P, P], BF16, kind="ExternalInput").ap()
    ones16 = nc.dram_tensor("ones16", [P, 16], BF16, kind="ExternalInput").ap()
    out = nc.dram_tensor("out", [T, E], BF16, kind="ExternalOutput").ap()
    denombuf = nc.dram_tensor("denombuf", [1, 8 * 2048], F32, kind="Internal").ap()
    recipbuf = nc.dram_tensor("recipbuf", [1, 8 * 2048], BF16, kind="Internal").ap()

    with tile.TileContext(nc) as tc, tc.tile_pool(name="data", bufs=1) as data, \
            tc.tile_pool(name="consts", bufs=1) as consts, \
            tc.tile_pool(name="qload", bufs=6) as qload, \
            tc.tile_pool(name="qload2", bufs=20) as qload2, \
            tc.tile_pool(name="expA", bufs=4) as expA, \
            tc.tile_pool(name="expB", bufs=26) as expB, \
            tc.tile_pool(name="scratch", bufs=3) as scratch, \
            tc.tile_pool(name="ostage", bufs=6) as ostage:
        qpT_c = [consts.tile([P, DQ], BF16, name=f"qp{ec}", tag=f"qp{ec}")
                 for ec in range(16)]
        kvpT_c = [consts.tile([P, P], BF16, name=f"kvp{ec}", tag=f"kvp{ec}")
                  for ec in range(16)]

        def load_weights(ec):
            nc.gpsimd.dma_start(out=kvpT_c[ec][:], in_=kvpT[ec * P:(ec + 1) * P, :])
            nc.gpsimd.dma_start(out=qpT_c[ec][:], in_=qpT[ec * P:(ec + 1) * P, :])

        load_weights(0)
        load_weights(1)
        qb_sb = consts.tile([P, 2], F32)
        nc.sync.dma_start(out=qb_sb[:], in_=qb)
        kvb_sb = consts.tile([P, 1], F32)
        nc.sync.dma_start(out=kvb_sb[:], in_=kvb)

        QT0 = data.tile([P, T], BF16)  # dq 0:128   (gi_loc 0, 1)
        QT1 = data.tile([P, T], BF16)  # dq 128:256 (gi_loc 2, 3)
        KVT = data.tile([P, T], BF16)  # k rows 0:64, v rows 64:128
        KTdup = data.tile([P, T], BF16)  # k rows duplicated at partitions 64:128
        attn0 = data.tile([P, T], BF16)  # attnoutT c-chunk 0 (gi_loc 0, 1)
        attn1 = data.tile([P, T], BF16)  # c-chunk 1 (gi_loc 2, 3)
        Vp = [data.tile([P, 16 * 65], BF16, name=f"vp{n}", tag=f"vp{n}") for n in range(N)]
        Vtmp = data.tile([64, T], BF16)
        opT_sb = [data.tile([P, E], BF16, name=f"opt{cc}", tag=f"opt{cc}") for cc in range(2)]
        identb = consts.tile([P, P], BF16)

        def late_consts():
            """Small consts needed by transposes/AV; issued mid-tchunk0.
            The 1MB opT load is deferred past all weight chunks so it never
            delays the head's weight feed (which stalls head matmuls, lets
            the scheduler hoist aux work, and pushes the evictions the
            first scores' psum banks WAR on ~35us late)."""
            nc.sync.dma_start(out=identb[:], in_=ident)
            for n in range(N):
                vcol = Vp[n].rearrange("p (m c) -> p m c", c=65)[:, :, 64:65]
                nc.sync.dma_start(out=vcol, in_=ones16)

        def load_opT():
            for cc in range(2):
                nc.sync.dma_start(out=opT_sb[cc][:], in_=opT[cc * P:(cc + 1) * P, :])

        # ---------------- Phase 1 bulk (batch 0) ----------------
        def p1_tchunk(tchunk, trans, ps1, psT, hook=None):
            tcols = slice(tchunk * 1024, (tchunk + 1) * 1024)
            pq0 = ps1.tile([P, 1024], F32, tag="pq0")
            pkv = ps1.tile([P, 1024], F32, tag="pkv")
            pq1 = ps1.tile([P, 1024], F32, tag="pq1")
            tj = 0
            for ec in range(16):
                qt = qload.tile([P, 1024], BF16, tag="qt")
                nc.gpsimd.dma_start(out=qt[:], in_=qT[ec * P:(ec + 1) * P, tcols])
                if hook is not None:
                    hook(ec)
                first, last = ec == 0, ec == 15
                for ps_t, w in ((pq0, qpT_c[ec][:, 0:P]),
                                (pq1, qpT_c[ec][:, P:DQ]),
                                (pkv, kvpT_c[ec][:])):
                    for lq in range(2):
                        nc.tensor.matmul(ps_t[:, lq * 512:(lq + 1) * 512], lhsT=w,
                                         rhs=qt[:, lq * 512:(lq + 1) * 512],
                                         start=first, stop=last)
                if ec % 2 == 1 and tj < len(trans):
                    tn, tmc = trans[tj]
                    tj += 1
                    do_transpose(tn, tmc, psT)
            nc.vector.tensor_scalar(QT0[:, tcols], pq0[:], qb_sb[:, 0:1], None, op0=add)
            nc.vector.tensor_scalar(QT1[:, tcols], pq1[:], qb_sb[:, 1:2], None, op0=add)
            nc.vector.tensor_scalar(KVT[:, tcols], pkv[:], kvb_sb[:, 0:1], None, op0=add)
            nc.sync.dma_start(out=KTdup[64:128, tcols], in_=KVT[0:64, tcols])
            nc.sync.dma_start(out=Vtmp[0:64, tcols], in_=KVT[64:128, tcols])

        def do_transpose(n, mc, psT):
            pt = psT.tile([P, 64], BF16, name="pt", tag="pt")
            nc.tensor.transpose(pt[:], Vtmp[0:64, n * L + mc * P:n * L + (mc + 1) * P],
                                identb[0:64, 0:64])
            nc.vector.tensor_copy(Vp[n][:, mc * 65:mc * 65 + 64], pt[:])

        # ---------------- aux items ----------------
        def item_p1_super(tchunk, colhalf):
            """All three projection targets over one 512-col span, sharing a
            single set of 16 qt loads (issued on the idle gpsimd queue).
            Order pkv, pq0, pq1 so KTdup/Vtmp are produced earliest."""
            lo = tchunk * 1024 + colhalf * 512
            qts = {}

            def ldq(ec):
                qts[ec] = qload2.tile([P, 512], BF16, name="qt2", tag="qt2")
                nc.gpsimd.dma_start(out=qts[ec][:],
                                    in_=qT[ec * P:(ec + 1) * P, lo:lo + 512])
            ev = []
            for ec in range(6):
                ev.append(('free', lambda ec=ec: ldq(ec)))
            for ti, target in enumerate((2, 0, 1)):
                cell = {}

                def alloc(cell=cell):
                    cell['t'] = CUR['apool'].tile()
                ev.append(('free', alloc))
                for ec in range(16):
                    if ti == 0 and ec + 6 < 16:
                        ev.append(('free', lambda ec=ec: ldq(ec + 6)))

                    def mm(ec=ec, target=target, cell=cell, last=ti == 2):
                        w = (qpT_c[ec][:, 0:P], qpT_c[ec][:, P:DQ],
                             kvpT_c[ec][:])[target]
                        q = qts.pop(ec) if last else qts[ec]
                        nc.tensor.matmul(cell['t'][:], lhsT=w, rhs=q[:],
                                         start=ec == 0, stop=ec == 15)
                    ev.append(('mm', mm))

                def evict(cell=cell, target=target):
                    dst = (QT0, QT1, KVT)[target]
                    bias = (qb_sb[:, 0:1], qb_sb[:, 1:2], kvb_sb[:, 0:1])[target]
                    nc.vector.tensor_scalar(dst[:, lo:lo + 512], cell['t'][:],
                                            bias, None, op0=add)
                    if target == 2:
                        nc.sync.dma_start(out=KTdup[64:128, lo:lo + 512],
                                          in_=KVT[0:64, lo:lo + 512])
                        nc.sync.dma_start(out=Vtmp[0:64, lo:lo + 512],
                                          in_=KVT[64:128, lo:lo + 512])
                ev.append(('free', evict))
            return ev

        def item_avB(n, pair, half, lq, ebs):
            """Deferred head-B AV over staged exp tiles, one 512-col pass."""
            attnp = attn0 if pair == 0 else attn1
            lo = n * L + half * 1024 + lq * 512
            seg = ((n * 2 + pair) * 2 + half) * 2048
            cell = {}

            def alloc():
                cell['t'] = CUR['apool'].tile()
            ev = [('free', alloc)]
            for mc in range(16):
                def mm(mc=mc):
                    vw = Vp[n][:, mc * 65:mc * 65 + 65]
                    nc.tensor.matmul(cell['t'][0:65, :], lhsT=vw,
                                     rhs=ebs[mc][:, lq * 512:(lq + 1) * 512],
                                     start=mc == 0, stop=mc == 15)
                ev.append(('mm', mm))

            def evict():
                sc = scratch.tile([64, 512], BF16, name="sc", tag="sc")
                nc.vector.tensor_copy(sc[:], cell['t'][0:64, :])
                nc.sync.dma_start(out=attnp[64:128, lo:lo + 512], in_=sc[:])
                dnB = scratch.tile([1, 512], F32, name="dnB", tag="dnB")
                nc.vector.tensor_copy(dnB[:], cell['t'][64:65, :])
                dB = seg + 1024 + lq * 512
                nc.sync.dma_start(out=denombuf[0:1, dB:dB + 512], in_=dnB[:])
            ev.append(('free', evict))
            return ev

        def item_p3(tt, eo, lq, evict_eng='dve'):
            """Output-projection 512-col tile: rows tt*128, e cols eo*1024+lq*512."""
            trows = slice(tt * P, (tt + 1) * P)
            ecol = eo * 1024 + lq * 512
            cell = {}

            def alloc():
                cell['t'] = CUR['apool'].tile()
            ev = [('free', alloc)]
            for cc in range(2):
                def mm(cc=cc):
                    src = attn0 if cc == 0 else attn1
                    nc.tensor.matmul(cell['t'][:], lhsT=src[:, trows],
                                     rhs=opT_sb[cc][:, ecol:ecol + 512],
                                     start=cc == 0, stop=cc == 1)
                ev.append(('mm', mm))

            def evict():
                ost = ostage.tile([P, 512], BF16, name="ost", tag="ost")
                if evict_eng == 'dve':
                    nc.vector.tensor_copy(ost[:], cell['t'][:])
                else:
                    nc.scalar.copy(ost[:], cell['t'][:])
                nc.sync.dma_start(out=out[trows, ecol:ecol + 512], in_=ost[:])
            ev.append(('free', evict))
            return ev

        def item_transposes(n, mcs, psT):
            return [('mm', lambda mc=mc: do_transpose(n, mc, psT)) for mc in mcs]

        def weave(base, extra, every=4):
            """Interleave `extra` events into `base`, one per `every` base
            events, so thin transposes never cluster on the PE."""
            outev, j = [], 0
            for k, e in enumerate(base):
                outev.append(e)
                if k % every == every - 1 and j < len(extra):
                    outev.append(extra[j])
                    j += 1
            outev.extend(extra[j:])
            return outev

        def ev_norm(n, pair, half):
            """Normalize attn rows for one (pair, half): 1024 l cols."""
            def norm():
                attnp = attn0 if pair == 0 else attn1
                lo = n * L + half * 1024
                seg = ((n * 2 + pair) * 2 + half) * 2048
                packed = scratch.tile([P, 16], F32, name="packed", tag="packed")
                nc.sync.dma_start(
                    out=packed[:],
                    in_=denombuf[0:1, seg:seg + 2048].rearrange("a (p c) -> (a p) c", p=P))
                recp = scratch.tile([P, 16], F32, name="recp", tag="recp")
                nc.vector.reciprocal(recp[:], packed[:])
                recb = scratch.tile([P, 16], BF16, name="recb", tag="recb")
                nc.vector.tensor_copy(recb[:], recp[:])
                nc.sync.dma_start(
                    out=recipbuf[0:1, seg:seg + 2048].rearrange("a (p c) -> (a p) c", p=P),
                    in_=recb[:])
                bct = scratch.tile([P, 1024], BF16, name="bct", tag="bct")
                nc.sync.dma_start(out=bct[0:64, :],
                                  in_=pbcast(recipbuf[0:1, seg:seg + 1024], 64))
                nc.sync.dma_start(out=bct[64:128, :],
                                  in_=pbcast(recipbuf[0:1, seg + 1024:seg + 2048], 64))
                nc.vector.tensor_mul(attnp[:, lo:lo + 1024], attnp[:, lo:lo + 1024],
                                     bct[:])
            return [('free', norm)]

        # ---------------- Phase 2 unit with aux interleave ----------------
        def p2_unit(n, pair, half, ps_s, ps_avA, aux, sink=None):
            QTp = QT0 if pair == 0 else QT1
            attnp = attn0 if pair == 0 else attn1
            lo = n * L + half * 1024
            seg = ((n * 2 + pair) * 2 + half) * 2048
            avA = ps_avA.tile([65, 1024], F32, tag="avA")
            ebs = []
            for mc in range(16):
                mo = n * L + mc * P
                sA = ps_s.tile([P, 1024], F32, tag="sA")
                sB = ps_s.tile([P, 1024], F32, tag="sB")
                for lq in range(2):
                    lc2 = slice(lo + lq * 512, lo + lq * 512 + 512)
                    nc.tensor.matmul(sA[:, lq * 512:(lq + 1) * 512],
                                     lhsT=KVT[0:64, mo:mo + P],
                                     rhs=QTp[0:64, lc2])
                    nc.tensor.matmul(sB[:, lq * 512:(lq + 1) * 512],
                                     lhsT=KTdup[64:128, mo:mo + P],
                                     rhs=QTp[64:128, lc2])
                eA = expA.tile([P, 1024], BF16, tag="eA")
                eB = expB.tile([P, 1024], BF16, tag="eB")
                nc.scalar.activation(eA[:], sA[:], mybir.ActivationFunctionType.Exp,
                                     scale=SCALE)
                nc.scalar.activation(eB[:], sB[:], mybir.ActivationFunctionType.Exp,
                                     scale=SCALE)
                ebs.append(eB)
                vw = Vp[n][:, mc * 65:mc * 65 + 65]
                for lq in range(2):
                    nc.tensor.matmul(avA[:, lq * 512:(lq + 1) * 512], lhsT=vw,
                                     rhs=eA[:, lq * 512:(lq + 1) * 512],
                                     start=mc == 0, stop=mc == 15)
                aux.pop(5)
            # head-A rows + denominator evict; head-B AV deferred to aux
            nc.vector.tensor_copy(attnp[0:64, lo:lo + 1024], avA[0:64, :])
            dnA = scratch.tile([1, 1024], F32, tag="dnA")
            nc.vector.tensor_copy(dnA[:], avA[64:65, :])
            nc.sync.dma_start(out=denombuf[0:1, seg:seg + 1024], in_=dnA[:])
            ev = (item_avB(n, pair, half, 0, ebs)
                  + item_avB(n, pair, half, 1, ebs)
                  + ev_norm(n, pair, half))
            if sink is None:
                aux.push_front(ev)
            else:
                sink.extend(ev)

        # ================= schedule =================
        aux = AuxQueue()
        with tc.tile_pool(name="ps1", bufs=1, space="PSUM") as ps1, \
                tc.tile_pool(name="psT0", bufs=2, space="PSUM") as psT0:
            p1_tchunk(0, [], ps1, psT0,
                      hook=lambda ec: (load_weights(ec + 2) if ec <= 13 else None,
                                       late_consts() if ec == 6 else None))
            p1_tchunk(1, [(0, mc) for mc in range(8)], ps1, psT0)
            load_opT()

        pending = []  # events deferred from n0's last unit into n1's queue
        for n in range(N):
            with tc.tile_pool(name=f"ps_s{n}", bufs=1, space="PSUM") as ps_s, \
                    tc.tile_pool(name=f"ps_avA{n}", bufs=1, space="PSUM") as ps_avA, \
                    tc.tile_pool(name=f"ps_aux{n}", bufs=1, space="PSUM") as psx, \
                    tc.tile_pool(name=f"psT_{n}", bufs=1, space="PSUM") as psT:
                if n == 0:
                    CUR['apool'] = AuxPool(psx, ["aux"])
                    # batch-1 projections as 4 super-nuggets; n0's leftover
                    # transposes weave into the first (avA needs them ~step 8)
                    # and each span's V' transposes weave into the next
                    tq = item_transposes(0, range(8, 16), psT)
                    for tchunk in (2, 3):
                        for colhalf in range(2):
                            aux.push(weave(item_p1_super(tchunk, colhalf), tq,
                                           every=2 if tchunk == 2 and
                                           colhalf == 0 else 8))
                            mc0 = (tchunk - 2) * 8 + colhalf * 4
                            tq = item_transposes(1, range(mc0, mc0 + 4), psT)
                    aux.push(tq)
                else:
                    CUR['apool'] = AuxPool(psx, ["aux", "aux2"])
                    aux.push(pending)
                    # P3 for batch 0 rides the queue inside P2(n1)
                    for tt in range(16):
                        for eo in range(2):
                            for lq in range(2):
                                aux.push(item_p3(tt, eo, lq))
                for pair in range(2):
                    for half in range(2):
                        last = n == 0 and pair == 1 and half == 1
                        p2_unit(n, pair, half, ps_s, ps_avA, aux,
                                sink=pending if last else None)
                        if n == 1 and pair == 1 and half == 0:
                            # first half of P3(n1) can ride once its rows norm
                            for tt in range(16, 24):
                                for eo in range(2):
                                    for lq in range(2):
                                        aux.push(item_p3(tt, eo, lq))
                aux.drain()
                if n == 1:
                    # tail: remaining P3 rows, same pools (no boundary gap)
                    for tt in range(24, 32):
                        for eo in range(2):
                            for lq in range(2):
                                for kind, fn in item_p3(tt, eo, lq,
                                                        'dve' if lq == 0
                                                        else 'act'):
                                    fn()

    nc.compile()
    return nc


_NC_CACHE = None


def _get_nc():
    global _NC_CACHE
    if _NC_CACHE is None:
        _NC_CACHE = build_nc()
    return _NC_CACHE


def make_in_maps(query, q_proj, q_bias, kv_proj, kv_bias, out_proj):
    """Host-side sharding. Returns list of 8 per-core input dicts."""
    qT_h = np.ascontiguousarray(
        np.asarray(query, dtype=np.float32).transpose(2, 1, 0).reshape(E, T)
    ).astype(ml_dtypes.bfloat16)
    q_proj = np.asarray(q_proj, dtype=np.float32)
    q_bias = np.asarray(q_bias, dtype=np.float32)
    kv_proj = np.asarray(kv_proj, dtype=np.float32)
    kv_bias = np.asarray(kv_bias, dtype=np.float32)
    out_proj = np.asarray(out_proj, dtype=np.float32)
    ident = np.eye(P, dtype=np.float32)

    in_maps = []
    for c in range(8):
        h0 = c // 2
        gis = range(4) if c % 2 == 0 else range(4, 8)
        rows_q = np.array([gi * (H * D) + h0 * D + d for gi in gis for d in range(D)])
        kv_rows = slice(h0 * 2 * D, (h0 + 1) * 2 * D)
        in_maps.append({
            "qT": qT_h,
            "qpT": np.ascontiguousarray(q_proj[rows_q, :].T).astype(ml_dtypes.bfloat16),
            "kvpT": np.ascontiguousarray(kv_proj[kv_rows, :].T).astype(ml_dtypes.bfloat16),
            "opT": np.ascontiguousarray(out_proj[:, rows_q].T).astype(ml_dtypes.bfloat16),
            "qb": np.ascontiguousarray(q_bias[rows_q].reshape(2, P).T),
            "kvb": np.ascontiguousarray(kv_bias[kv_rows].reshape(P, 1)),
            "ident": ident.astype(ml_dtypes.bfloat16),
            "ones16": np.ones((P, 16), dtype=ml_dtypes.bfloat16),
        })
    return in_maps


def kernel(query, q_proj, q_bias, kv_proj, kv_bias, out_proj, out_bias):
    from concourse.bass_utils import run_bass_kernel_spmd

    nc = _get_nc()
    in_maps = make_in_maps(query, q_proj, q_bias, kv_proj, kv_bias, out_proj)
    res = run_bass_kernel_spmd(nc, in_maps, core_ids=list(range(8)))
    total = np.zeros((T, E), dtype=np.float64)
    for rmap in res.results:
        total += rmap["out"].astype(np.float64)
    total += np.asarray(out_bias, dtype=np.float64)[None, :]
    return np.ascontiguousarray(
        total.reshape(N, L, E).transpose(1, 0, 2)).astype(np.float32)
